# revision 18
# baseline (speedup 1.0000x reference)
"""MoE layer (top-2 of 8 experts, SwiGLU) on 8 Trainium2 NeuronCores.

Strategy (expert-parallel, matching the sharding hint):
  - Host computes the router (gate logits -> top-2 -> softmax) in fp32 numpy,
    exactly mirroring the reference math, and gathers each expert's tokens
    (the all-to-all dispatch). Each core gets one expert's weights + tokens.
  - Each core runs a dense SwiGLU MLP over its gathered token batch:
        h = silu(x @ w_gate.T) * (x @ w1.T);  y = h @ w2.T
    in bf16 with fp32 PSUM accumulation.
  - Host multiplies outputs by the combine weights and scatter-adds back
    into token order (the combine step).

Kernel schedule (v2, fully SBUF-resident weights):
  All three weight matrices live in SBUF (3 x 64 KB/partition = 192 KB of
  the ~208 KB budget), so steady state has ZERO weight DMA -- only x in and
  y out. Tokens are processed in blocks of 256 (tail 128). Within a block,
  stage 1 (the two [*,1024]x[1024,4096] matmuls + silu*mul) and stage 2
  (h @ w2.T) are interleaved PER dff-TILE: h[df] is consumed by stage-2
  matmuls immediately after it is produced, so only a handful of h tiles
  are ever live. PSUM: 2 psg + 2 ps1t (full banks, double-buffered against
  the Act/DVE readers) + 4 psys accumulators (2 token-tiles x 2 d-halves)
  = exactly 8 banks.

DRAM layouts are pre-swizzled on host so every DMA is a >=4KB-contiguous-
per-partition copy:
  xt  [128, KD*C]    : x gathered+transposed, block-major
                       (block b cols [b*KD*256 ...], within: d*tb + j)
  w1r/wgr [128, 8, 4096] : weight chunk dfc contiguous; within a chunk
                       cols = d*512 + (dff % 512)
  w2r [128, 8, 4096] : w2.T chunk c covers df in [4c, 4c+4); within:
                       (df%4)*1024 + dcol
  y   [C, 1024] bf16 : output token-major (host casts to fp32)
"""

import numpy as np
import ml_dtypes

import concourse.bass as bass
import concourse.mybir as mybir
import concourse.tile as tile
from concourse.bass_utils import run_bass_kernel_spmd

# ---------------------------------------------------------------------------
# Workaround for this walrus build: TPB instructions have a single hardware
# wait slot and this walrus refuses any instruction carrying more than one
# sem wait ("Too many sync wait commands"). Post-pass: for every instruction
# with k>1 waits, hoist k-1 waits onto single-wait NOPs on the same engine
# immediately before it. Program-order semantics are identical (the engine
# blocks on each wait in turn before issuing the instruction).
# ---------------------------------------------------------------------------

_ws_counter = [0]


def _split_multi_waits(nc: bass.Bass) -> int:
    n_split = 0
    for f in nc.m.functions:
        for bb in f.blocks:
            new_insts = []
            for inst in bb.instructions:
                si = inst.sync_info
                if si is not None and si.on_wait and len(si.on_wait) > 1:
                    waits = list(si.on_wait)
                    for w in waits[:-1]:
                        _ws_counter[0] += 1
                        n_split += 1
                        new_insts.append(
                            mybir.InstNoOp(
                                name=f"waitsplit-{_ws_counter[0]}",
                                opcode="NoOp",
                                engine=inst.engine,
                                sync_info=mybir.SyncInfo(
                                    on_wait=[w], on_update=[]
                                ),
                                bass_nofuse=True,
                                text_hint="waitsplit",
                            )
                        )
                    si.on_wait = [waits[-1]]
                new_insts.append(inst)
            bb.instructions[:] = new_insts
    return n_split

# ---------------------------------------------------------------------------

D = 1024
DFF = 4096
N_EXPERTS = 8
TOP_K = 2
N_CORES = 8
TB = 256          # token block size (m-tiles per block = TB//128 = 2)
KD = D // 128     # 8 contraction tiles over d
NF = DFF // 128   # 32 dff tiles
NCH = NF // 4     # 8 weight chunks (512 dff cols each)

XT_BUFS = 2
HG_BUFS = 3
H_BUFS = 6
YO_BUFS = 2

BF16 = mybir.dt.bfloat16
F32 = mybir.dt.float32
NP_BF16 = ml_dtypes.bfloat16

_NC_CACHE: dict[int, bass.Bass] = {}


def _plan_blocks(C: int) -> list[int]:
    """Token-block plan: blocks of <=256 tokens (psys needs 2*ceil(tb/128)
    PSUM banks <= 4). Avoid a thin 128-token tail by evening out the last
    two blocks (N=128 matmuls can't hide their LDWEIGHTS)."""
    assert C % 128 == 0
    blocks = [TB] * (C // TB)
    if C % TB:
        if blocks:
            blocks = blocks[:-1] + [192, 192]
        else:
            blocks = [128]
    return blocks


def _build_kernel(C: int, repeat: int = 1) -> bass.Bass:
    """Dense SwiGLU MLP over C tokens (C a multiple of 128).

    repeat>1 wraps the computation in a hardware For_i loop (weights are
    preloaded outside the loop) -- used for wall-clock calibration."""
    blocks = _plan_blocks(C)

    nc = bass.Bass()
    xt = nc.dram_tensor("xt", [128, KD * C], BF16, kind="ExternalInput")
    w1r = nc.dram_tensor("w1r", [128, NCH, 4096], BF16, kind="ExternalInput")
    wgr = nc.dram_tensor("wgr", [128, NCH, 4096], BF16, kind="ExternalInput")
    w2r = nc.dram_tensor("w2r", [128, NCH, 4096], BF16, kind="ExternalInput")
    y = nc.dram_tensor("y", [C, D], BF16, kind="ExternalOutput")

    silu = mybir.ActivationFunctionType.Silu

    with tile.TileContext(nc) as tc:
        with (
            tc.tile_pool(name="wres", bufs=1) as wres,
            tc.tile_pool(name="xt", bufs=XT_BUFS) as xtpool,
            tc.tile_pool(name="hg", bufs=HG_BUFS) as hgpool,
            tc.tile_pool(name="h", bufs=H_BUFS) as hpool,
            tc.tile_pool(name="yo", bufs=YO_BUFS) as ypool,
            tc.tile_pool(name="ps1", bufs=1, space="PSUM") as psum1,
            tc.tile_pool(name="ps2", bufs=4, space="PSUM") as psum2,
        ):
            # Resident weights: per-chunk tiles (8 KB/partition each).
            wg_t = [wres.tile([128, 4096], BF16, name=f"wg{c}") for c in range(NCH)]
            w1_t = [wres.tile([128, 4096], BF16, name=f"w1{c}") for c in range(NCH)]
            w2_t = [wres.tile([128, 4096], BF16, name=f"w2{c}") for c in range(NCH)]

            def load_weights(jit=False):
                # Consumption order for block 0: stage-1 uses wg/w1 chunk
                # dfc piece j at df=4*dfc+j; stage-2 needs the w2 piece one
                # df later. In jit (single-shot) mode the stream goes over
                # the otherwise-idle GpSimd SWDGE ring so it contends with
                # neither xt/y (SP HWDGE) nor silu issue (Act), and the
                # first wg/w1 chunk is split so block 0 starts after 256KB.
                eng = nc.gpsimd if jit else nc.scalar
                for c in range(NCH):
                    if jit and c == 0:
                        for j in range(4):
                            s = slice(j * 1024, (j + 1) * 1024)
                            eng.dma_start(wg_t[0][:, s], wgr[:, 0, s])
                            eng.dma_start(w1_t[0][:, s], w1r[:, 0, s])
                        eng.dma_start(w2_t[0][:], w2r[:, 0, :])
                    else:
                        eng.dma_start(wg_t[c][:], wgr[:, c, :])
                        eng.dma_start(w1_t[c][:], w1r[:, c, :])
                        eng.dma_start(w2_t[c][:], w2r[:, c, :])

            if repeat > 1:
                load_weights()

            def body():
                tok0 = 0
                xt_next = None
                prev = None
                for b, tb in enumerate(blocks):
                    n_m = -(-tb // 128)
                    m_w = [min(128, tb - m * 128) for m in range(n_m)]
                    if xt_next is None:
                        xt_sb = xtpool.tile([128, KD * tb], BF16, tag="xt")
                        nc.sync.dma_start(
                            xt_sb[:], xt[:, tok0 * KD:(tok0 + tb) * KD]
                        )
                    else:
                        xt_sb = xt_next
                    if b + 1 < len(blocks):
                        nb = blocks[b + 1]
                        nt0 = tok0 + tb
                        xt_next = xtpool.tile([128, KD * nb], BF16, tag="xt")
                        nc.sync.dma_start(
                            xt_next[:], xt[:, nt0 * KD:(nt0 + nb) * KD]
                        )
                    else:
                        xt_next = None
                    if b == 0 and repeat == 1:
                        load_weights(jit=True)
                    psys = [
                        psum2.tile([128, 512], F32, tag="psy", name=f"psy{m}_{h}")
                        for m in range(n_m) for h in range(2)
                    ]

                    def s2(df, h, psys_l, m_w_l):
                        # stage-2: consume h[df] into the psys accumulators
                        dfc = df // 4
                        for m, mw in enumerate(m_w_l):
                            for half in range(2):
                                nc.tensor.matmul(
                                    psys_l[m * 2 + half][:mw, :],
                                    h[:, m * 128:m * 128 + mw],
                                    w2_t[dfc][:, (df % 4) * 1024 + half * 512:
                                              (df % 4) * 1024 + (half + 1) * 512],
                                    start=(df == 0),
                                    stop=(df == NF - 1),
                                )

                    def flush_prev():
                        # stage-2 of the previous block's last df + its
                        # psys evacuation, deferred into this block so the
                        # PE never waits on h[31] at a block boundary.
                        nonlocal prev
                        if prev is None:
                            return
                        p_h, p_psys, p_mw, p_tok0 = prev
                        s2(NF - 1, p_h, p_psys, p_mw)
                        for m, mw in enumerate(p_mw):
                            for half in range(2):
                                yo = ypool.tile([128, 512], BF16, tag="yo")
                                nc.vector.tensor_copy(
                                    yo[:mw, :], p_psys[m * 2 + half][:mw, :]
                                )
                                nc.sync.dma_start(
                                    y[p_tok0 + m * 128:p_tok0 + m * 128 + mw,
                                      half * 512:(half + 1) * 512],
                                    yo[:mw, :],
                                )
                        prev = None

                    pending = None
                    for df in range(NF):
                        dfc, j = df // 4, df % 4
                        psg = psum1.tile([128, 512], F32, tag="psg", bufs=2)
                        for d in range(KD):
                            nc.tensor.matmul(
                                psg[:, :tb],
                                wg_t[dfc][:, j * 1024 + d * 128:
                                          j * 1024 + (d + 1) * 128],
                                xt_sb[:, d * tb:(d + 1) * tb],
                                start=(d == 0),
                                stop=(d == KD - 1),
                            )
                        ps1 = psum1.tile([128, 512], F32, tag="ps1", bufs=2)
                        for d in range(KD):
                            nc.tensor.matmul(
                                ps1[:, :tb],
                                w1_t[dfc][:, j * 1024 + d * 128:
                                          j * 1024 + (d + 1) * 128],
                                xt_sb[:, d * tb:(d + 1) * tb],
                                start=(d == 0),
                                stop=(d == KD - 1),
                            )
                        hg = hgpool.tile([128, tb], BF16, tag="hg")
                        nc.scalar.activation(hg[:], psg[:, :tb], silu)
                        h = hpool.tile([128, tb], BF16, tag="h")
                        nc.vector.tensor_mul(h[:], hg[:], ps1[:, :tb])
                        # software pipeline: emit stage-2 of df-1 after
                        # stage-1 of df, so the in-order PE queue never
                        # waits on Act/DVE producing h[df]. The previous
                        # block's last-df stage-2 + psys copies land after
                        # this block's first stage-1 group.
                        if df == 1:
                            flush_prev()
                        if pending is not None:
                            s2(df - 1, pending, psys, m_w)
                        pending = h
                    prev = (pending, psys, m_w, tok0)
                    tok0 += tb
                flush_prev()

            if repeat == 1:
                body()
            else:
                with tc.For_i(0, repeat, 1):
                    body()
    _split_multi_waits(nc)
    return nc


def _swizzle_k(a: np.ndarray) -> np.ndarray:
    """[K, F] -> [128, K//128, F] with K = ko*128 + p on partitions."""
    k, f = a.shape
    return np.ascontiguousarray(
        a.reshape(k // 128, 128, f).transpose(1, 0, 2)
    )


def _pack_w1(w: np.ndarray) -> np.ndarray:
    """w [DFF, D] (w1[e] or w_gate[e]) -> [128, NCH, 4096] bf16, j-major:

    chunk dfc, col j*1024 + d*128 + r  ==  w.T swizzled [p, d,
    dfc*512 + j*128 + r] -- so the df=(4*dfc+j) stage-1 weights are one
    contiguous 256KB piece of chunk dfc."""
    wt = _swizzle_k(np.ascontiguousarray(w.T).astype(np.float32))  # [128, KD, DFF]
    out = wt.reshape(128, KD, NCH, 4, 128).transpose(0, 2, 3, 1, 4)
    return np.ascontiguousarray(
        out.reshape(128, NCH, 4096)
    ).astype(NP_BF16)


def _pack_w2(w2: np.ndarray) -> np.ndarray:
    """w2 [D, DFF] -> [128, NCH, 4096] bf16.

    chunk c covers df in [4c, 4c+4); within: (df%4)*1024 + dcol."""
    wt = _swizzle_k(np.ascontiguousarray(w2.T).astype(np.float32))  # [128, NF, D]
    return np.ascontiguousarray(
        wt.reshape(128, NCH, 4 * D)
    ).astype(NP_BF16)


def _pack_xt(xf_sel: np.ndarray, C: int) -> np.ndarray:
    """Gathered tokens [n, D] -> [128, KD*C] bf16, block-major."""
    n = xf_sel.shape[0]
    xt_full = np.zeros((D, C), dtype=np.float32)
    xt_full[:, :n] = xf_sel.T
    sw = _swizzle_k(xt_full)                    # [128, KD, C]
    blocks = _plan_blocks(C)
    parts = []
    t0 = 0
    for tb in blocks:
        parts.append(sw[:, :, t0:t0 + tb].reshape(128, KD * tb))
        t0 += tb
    return np.ascontiguousarray(np.concatenate(parts, axis=1)).astype(NP_BF16)


def make_in_maps(x, gate_w, w1, w_gate, w2):
    """Host router + dispatch. Returns (in_maps, sels, pair_w, C, n_tok)."""
    b, t, d = x.shape
    xf = np.ascontiguousarray(x.reshape(-1, d)).astype(np.float32)
    n_tok = xf.shape[0]

    logits = xf @ gate_w.T.astype(np.float32)                       # [N, E]
    top_idx = np.argsort(-logits, axis=1, kind="stable")[:, :TOP_K]
    top_vals = np.take_along_axis(logits, top_idx, axis=1)
    m = top_vals.max(axis=1, keepdims=True)
    ex = np.exp(top_vals - m)
    top_w = ex / ex.sum(axis=1, keepdims=True)

    pair_expert = top_idx.reshape(-1)
    pair_w = top_w.reshape(-1).astype(np.float32)
    order = np.argsort(pair_expert, kind="stable")
    counts = np.bincount(pair_expert, minlength=N_EXPERTS)
    starts = np.concatenate([[0], np.cumsum(counts)])

    C = max(TB, int(-(-int(counts.max()) // 128)) * 128)

    in_maps = []
    sels = []
    for e in range(N_EXPERTS):
        sel = order[starts[e]:starts[e + 1]]
        sels.append(sel)
        tok = sel // TOP_K
        in_maps.append(
            {
                "xt": _pack_xt(xf[tok], C),
                "w1r": _pack_w1(w1[e]),
                "wgr": _pack_w1(w_gate[e]),
                "w2r": _pack_w2(w2[e]),
            }
        )
    return in_maps, sels, pair_w, C, n_tok


def kernel(x, gate_w, w1, w_gate, w2):
    b, t, d = x.shape
    in_maps, sels, pair_w, C, n_tok = make_in_maps(x, gate_w, w1, w_gate, w2)

    if C not in _NC_CACHE:
        _NC_CACHE[C] = _build_kernel(C)
    nc = _NC_CACHE[C]

    res = run_bass_kernel_spmd(nc, in_maps, core_ids=list(range(N_CORES)))

    # Combine (host): weight by router prob, scatter-add to token order.
    contrib = np.zeros((n_tok * TOP_K, D), dtype=np.float32)
    for e in range(N_EXPERTS):
        sel = sels[e]
        y_e = res.results[e]["y"][: len(sel)].astype(np.float32)
        contrib[sel] = y_e * pair_w[sel][:, None]
    out = contrib.reshape(n_tok, TOP_K, D).sum(axis=1)
    return out.reshape(b, t, d).astype(x.dtype)


# revision 19
# speedup vs baseline: 1.0413x; 1.0413x over previous
"""MoE layer (top-2 of 8 experts, SwiGLU) on 8 Trainium2 NeuronCores.

Strategy (expert-parallel, matching the sharding hint):
  - Host computes the router (gate logits -> top-2 -> softmax) in fp32 numpy,
    exactly mirroring the reference math, and gathers each expert's tokens
    (the all-to-all dispatch). Each core gets one expert's weights + tokens.
  - Each core runs a dense SwiGLU MLP over its gathered token batch:
        h = silu(x @ w_gate.T) * (x @ w1.T);  y = h @ w2.T
    in bf16 with fp32 PSUM accumulation.
  - Host multiplies outputs by the combine weights and scatter-adds back
    into token order (the combine step).

Kernel schedule (v2, fully SBUF-resident weights):
  All three weight matrices live in SBUF (3 x 64 KB/partition = 192 KB of
  the ~208 KB budget), so steady state has ZERO weight DMA -- only x in and
  y out. Tokens are processed in blocks of 256 (tail 128). Within a block,
  stage 1 (the two [*,1024]x[1024,4096] matmuls + silu*mul) and stage 2
  (h @ w2.T) are interleaved PER dff-TILE: h[df] is consumed by stage-2
  matmuls immediately after it is produced, so only a handful of h tiles
  are ever live. PSUM: 2 psg + 2 ps1t (full banks, double-buffered against
  the Act/DVE readers) + 4 psys accumulators (2 token-tiles x 2 d-halves)
  = exactly 8 banks.

DRAM layouts are pre-swizzled on host so every DMA is a >=4KB-contiguous-
per-partition copy:
  xt  [128, KD*C]    : x gathered+transposed, block-major
                       (block b cols [b*KD*256 ...], within: d*tb + j)
  w1r/wgr [128, 8, 4096] : weight chunk dfc contiguous; within a chunk
                       cols = d*512 + (dff % 512)
  w2r [128, 8, 4096] : w2.T chunk c covers df in [4c, 4c+4); within:
                       (df%4)*1024 + dcol
  y   [C, 1024] bf16 : output token-major (host casts to fp32)
"""

import numpy as np
import ml_dtypes

import concourse.bass as bass
import concourse.mybir as mybir
import concourse.tile as tile
from concourse.bass_utils import run_bass_kernel_spmd

# ---------------------------------------------------------------------------
# Workaround for this walrus build: TPB instructions have a single hardware
# wait slot and this walrus refuses any instruction carrying more than one
# sem wait ("Too many sync wait commands"). Post-pass: for every instruction
# with k>1 waits, hoist k-1 waits onto single-wait NOPs on the same engine
# immediately before it. Program-order semantics are identical (the engine
# blocks on each wait in turn before issuing the instruction).
# ---------------------------------------------------------------------------

_ws_counter = [0]


def _split_multi_waits(nc: bass.Bass) -> int:
    n_split = 0
    for f in nc.m.functions:
        for bb in f.blocks:
            new_insts = []
            for inst in bb.instructions:
                si = inst.sync_info
                if si is not None and si.on_wait and len(si.on_wait) > 1:
                    waits = list(si.on_wait)
                    for w in waits[:-1]:
                        _ws_counter[0] += 1
                        n_split += 1
                        new_insts.append(
                            mybir.InstNoOp(
                                name=f"waitsplit-{_ws_counter[0]}",
                                opcode="NoOp",
                                engine=inst.engine,
                                sync_info=mybir.SyncInfo(
                                    on_wait=[w], on_update=[]
                                ),
                                bass_nofuse=True,
                                text_hint="waitsplit",
                            )
                        )
                    si.on_wait = [waits[-1]]
                new_insts.append(inst)
            bb.instructions[:] = new_insts
    return n_split

# ---------------------------------------------------------------------------

D = 1024
DFF = 4096
N_EXPERTS = 8
TOP_K = 2
N_CORES = 8
TB = 256          # token block size (m-tiles per block = TB//128 = 2)
KD = D // 128     # 8 contraction tiles over d
NF = DFF // 128   # 32 dff tiles
NCH = NF // 4     # 8 weight chunks (512 dff cols each)

XT_BUFS = 2
HG_BUFS = 3
H_BUFS = 6
YO_BUFS = 2

BF16 = mybir.dt.bfloat16
F32 = mybir.dt.float32
NP_BF16 = ml_dtypes.bfloat16

_NC_CACHE: dict[int, bass.Bass] = {}


def _plan_blocks(C: int) -> list[int]:
    """Token-block plan: blocks of <=256 tokens (psys needs 2*ceil(tb/128)
    PSUM banks <= 4). A 128-token tail beats evening out to 192+192: every
    m-tile costs a full N=512 stage-2 stream regardless of its width, so
    fewer m-tiles wins (measured 725.8us vs 756.6us)."""
    assert C % 128 == 0
    blocks = [TB] * (C // TB)
    if C % TB:
        blocks.append(128)
    return blocks


def _build_kernel(C: int, repeat: int = 1) -> bass.Bass:
    """Dense SwiGLU MLP over C tokens (C a multiple of 128).

    repeat>1 wraps the computation in a hardware For_i loop (weights are
    preloaded outside the loop) -- used for wall-clock calibration."""
    blocks = _plan_blocks(C)

    nc = bass.Bass()
    xt = nc.dram_tensor("xt", [128, KD * C], BF16, kind="ExternalInput")
    w1r = nc.dram_tensor("w1r", [128, NCH, 4096], BF16, kind="ExternalInput")
    wgr = nc.dram_tensor("wgr", [128, NCH, 4096], BF16, kind="ExternalInput")
    w2r = nc.dram_tensor("w2r", [128, NCH, 4096], BF16, kind="ExternalInput")
    y = nc.dram_tensor("y", [C, D], BF16, kind="ExternalOutput")

    silu = mybir.ActivationFunctionType.Silu

    with tile.TileContext(nc) as tc:
        with (
            tc.tile_pool(name="wres", bufs=1) as wres,
            tc.tile_pool(name="xt", bufs=XT_BUFS) as xtpool,
            tc.tile_pool(name="hg", bufs=HG_BUFS) as hgpool,
            tc.tile_pool(name="h", bufs=H_BUFS) as hpool,
            tc.tile_pool(name="yo", bufs=YO_BUFS) as ypool,
            tc.tile_pool(name="ps1", bufs=1, space="PSUM") as psum1,
            tc.tile_pool(name="ps2", bufs=4, space="PSUM") as psum2,
        ):
            # Resident weights: per-chunk tiles (8 KB/partition each).
            wg_t = [wres.tile([128, 4096], BF16, name=f"wg{c}") for c in range(NCH)]
            w1_t = [wres.tile([128, 4096], BF16, name=f"w1{c}") for c in range(NCH)]
            w2_t = [wres.tile([128, 4096], BF16, name=f"w2{c}") for c in range(NCH)]

            def load_weights(jit=False):
                # Consumption order for block 0: stage-1 uses wg/w1 chunk
                # dfc piece j at df=4*dfc+j; stage-2 needs the w2 piece one
                # df later. In jit (single-shot) mode the stream goes over
                # the otherwise-idle GpSimd SWDGE ring so it contends with
                # neither xt/y (SP HWDGE) nor silu issue (Act), and the
                # first wg/w1 chunk is split so block 0 starts after 256KB.
                eng = nc.gpsimd if jit else nc.scalar
                for c in range(NCH):
                    if jit and c == 0:
                        for j in range(4):
                            s = slice(j * 1024, (j + 1) * 1024)
                            eng.dma_start(wg_t[0][:, s], wgr[:, 0, s])
                            eng.dma_start(w1_t[0][:, s], w1r[:, 0, s])
                        eng.dma_start(w2_t[0][:], w2r[:, 0, :])
                    else:
                        eng.dma_start(wg_t[c][:], wgr[:, c, :])
                        eng.dma_start(w1_t[c][:], w1r[:, c, :])
                        eng.dma_start(w2_t[c][:], w2r[:, c, :])

            if repeat > 1:
                load_weights()

            def body():
                tok0 = 0
                xt_next = None
                prev = None
                for b, tb in enumerate(blocks):
                    n_m = -(-tb // 128)
                    m_w = [min(128, tb - m * 128) for m in range(n_m)]
                    if xt_next is None:
                        xt_sb = xtpool.tile([128, KD * tb], BF16, tag="xt")
                        nc.sync.dma_start(
                            xt_sb[:], xt[:, tok0 * KD:(tok0 + tb) * KD]
                        )
                    else:
                        xt_sb = xt_next
                    if b + 1 < len(blocks):
                        nb = blocks[b + 1]
                        nt0 = tok0 + tb
                        xt_next = xtpool.tile([128, KD * nb], BF16, tag="xt")
                        nc.sync.dma_start(
                            xt_next[:], xt[:, nt0 * KD:(nt0 + nb) * KD]
                        )
                    else:
                        xt_next = None
                    if b == 0 and repeat == 1:
                        load_weights(jit=True)
                    psys = [
                        psum2.tile([128, 512], F32, tag="psy", name=f"psy{m}_{h}")
                        for m in range(n_m) for h in range(2)
                    ]

                    def s2(df, h, psys_l, m_w_l):
                        # stage-2: consume h[df] into the psys accumulators
                        dfc = df // 4
                        for m, mw in enumerate(m_w_l):
                            for half in range(2):
                                nc.tensor.matmul(
                                    psys_l[m * 2 + half][:mw, :],
                                    h[:, m * 128:m * 128 + mw],
                                    w2_t[dfc][:, (df % 4) * 1024 + half * 512:
                                              (df % 4) * 1024 + (half + 1) * 512],
                                    start=(df == 0),
                                    stop=(df == NF - 1),
                                )

                    def flush_prev():
                        # stage-2 of the previous block's last df + its
                        # psys evacuation, deferred into this block so the
                        # PE never waits on h[31] at a block boundary.
                        nonlocal prev
                        if prev is None:
                            return
                        p_h, p_psys, p_mw, p_tok0 = prev
                        s2(NF - 1, p_h, p_psys, p_mw)
                        for m, mw in enumerate(p_mw):
                            for half in range(2):
                                yo = ypool.tile([128, 512], BF16, tag="yo")
                                nc.vector.tensor_copy(
                                    yo[:mw, :], p_psys[m * 2 + half][:mw, :]
                                )
                                nc.sync.dma_start(
                                    y[p_tok0 + m * 128:p_tok0 + m * 128 + mw,
                                      half * 512:(half + 1) * 512],
                                    yo[:mw, :],
                                )
                        prev = None

                    pending = None
                    for df in range(NF):
                        dfc, j = df // 4, df % 4
                        psg = psum1.tile([128, 512], F32, tag="psg", bufs=2)
                        for d in range(KD):
                            nc.tensor.matmul(
                                psg[:, :tb],
                                wg_t[dfc][:, j * 1024 + d * 128:
                                          j * 1024 + (d + 1) * 128],
                                xt_sb[:, d * tb:(d + 1) * tb],
                                start=(d == 0),
                                stop=(d == KD - 1),
                            )
                        ps1 = psum1.tile([128, 512], F32, tag="ps1", bufs=2)
                        for d in range(KD):
                            nc.tensor.matmul(
                                ps1[:, :tb],
                                w1_t[dfc][:, j * 1024 + d * 128:
                                          j * 1024 + (d + 1) * 128],
                                xt_sb[:, d * tb:(d + 1) * tb],
                                start=(d == 0),
                                stop=(d == KD - 1),
                            )
                        hg = hgpool.tile([128, tb], BF16, tag="hg")
                        nc.scalar.activation(hg[:], psg[:, :tb], silu)
                        h = hpool.tile([128, tb], BF16, tag="h")
                        nc.vector.tensor_mul(h[:], hg[:], ps1[:, :tb])
                        # software pipeline: emit stage-2 of df-1 after
                        # stage-1 of df, so the in-order PE queue never
                        # waits on Act/DVE producing h[df]. The previous
                        # block's last-df stage-2 + psys copies land after
                        # this block's first stage-1 group.
                        if df == 1:
                            flush_prev()
                        if pending is not None:
                            s2(df - 1, pending, psys, m_w)
                        pending = h
                    prev = (pending, psys, m_w, tok0)
                    tok0 += tb
                flush_prev()

            if repeat == 1:
                body()
            else:
                with tc.For_i(0, repeat, 1):
                    body()
    _split_multi_waits(nc)
    return nc


def _swizzle_k(a: np.ndarray) -> np.ndarray:
    """[K, F] -> [128, K//128, F] with K = ko*128 + p on partitions."""
    k, f = a.shape
    return np.ascontiguousarray(
        a.reshape(k // 128, 128, f).transpose(1, 0, 2)
    )


def _pack_w1(w: np.ndarray) -> np.ndarray:
    """w [DFF, D] (w1[e] or w_gate[e]) -> [128, NCH, 4096] bf16, j-major:

    chunk dfc, col j*1024 + d*128 + r  ==  w.T swizzled [p, d,
    dfc*512 + j*128 + r] -- so the df=(4*dfc+j) stage-1 weights are one
    contiguous 256KB piece of chunk dfc."""
    wt = _swizzle_k(np.ascontiguousarray(w.T).astype(np.float32))  # [128, KD, DFF]
    out = wt.reshape(128, KD, NCH, 4, 128).transpose(0, 2, 3, 1, 4)
    return np.ascontiguousarray(
        out.reshape(128, NCH, 4096)
    ).astype(NP_BF16)


def _pack_w2(w2: np.ndarray) -> np.ndarray:
    """w2 [D, DFF] -> [128, NCH, 4096] bf16.

    chunk c covers df in [4c, 4c+4); within: (df%4)*1024 + dcol."""
    wt = _swizzle_k(np.ascontiguousarray(w2.T).astype(np.float32))  # [128, NF, D]
    return np.ascontiguousarray(
        wt.reshape(128, NCH, 4 * D)
    ).astype(NP_BF16)


def _pack_xt(xf_sel: np.ndarray, C: int) -> np.ndarray:
    """Gathered tokens [n, D] -> [128, KD*C] bf16, block-major."""
    n = xf_sel.shape[0]
    xt_full = np.zeros((D, C), dtype=np.float32)
    xt_full[:, :n] = xf_sel.T
    sw = _swizzle_k(xt_full)                    # [128, KD, C]
    blocks = _plan_blocks(C)
    parts = []
    t0 = 0
    for tb in blocks:
        parts.append(sw[:, :, t0:t0 + tb].reshape(128, KD * tb))
        t0 += tb
    return np.ascontiguousarray(np.concatenate(parts, axis=1)).astype(NP_BF16)


def make_in_maps(x, gate_w, w1, w_gate, w2):
    """Host router + dispatch. Returns (in_maps, sels, pair_w, C, n_tok)."""
    b, t, d = x.shape
    xf = np.ascontiguousarray(x.reshape(-1, d)).astype(np.float32)
    n_tok = xf.shape[0]

    logits = xf @ gate_w.T.astype(np.float32)                       # [N, E]
    top_idx = np.argsort(-logits, axis=1, kind="stable")[:, :TOP_K]
    top_vals = np.take_along_axis(logits, top_idx, axis=1)
    m = top_vals.max(axis=1, keepdims=True)
    ex = np.exp(top_vals - m)
    top_w = ex / ex.sum(axis=1, keepdims=True)

    pair_expert = top_idx.reshape(-1)
    pair_w = top_w.reshape(-1).astype(np.float32)
    order = np.argsort(pair_expert, kind="stable")
    counts = np.bincount(pair_expert, minlength=N_EXPERTS)
    starts = np.concatenate([[0], np.cumsum(counts)])

    C = max(TB, int(-(-int(counts.max()) // 128)) * 128)

    in_maps = []
    sels = []
    for e in range(N_EXPERTS):
        sel = order[starts[e]:starts[e + 1]]
        sels.append(sel)
        tok = sel // TOP_K
        in_maps.append(
            {
                "xt": _pack_xt(xf[tok], C),
                "w1r": _pack_w1(w1[e]),
                "wgr": _pack_w1(w_gate[e]),
                "w2r": _pack_w2(w2[e]),
            }
        )
    return in_maps, sels, pair_w, C, n_tok


def kernel(x, gate_w, w1, w_gate, w2):
    b, t, d = x.shape
    in_maps, sels, pair_w, C, n_tok = make_in_maps(x, gate_w, w1, w_gate, w2)

    if C not in _NC_CACHE:
        _NC_CACHE[C] = _build_kernel(C)
    nc = _NC_CACHE[C]

    res = run_bass_kernel_spmd(nc, in_maps, core_ids=list(range(N_CORES)))

    # Combine (host): weight by router prob, scatter-add to token order.
    contrib = np.zeros((n_tok * TOP_K, D), dtype=np.float32)
    for e in range(N_EXPERTS):
        sel = sels[e]
        y_e = res.results[e]["y"][: len(sel)].astype(np.float32)
        contrib[sel] = y_e * pair_w[sel][:, None]
    out = contrib.reshape(n_tok, TOP_K, D).sum(axis=1)
    return out.reshape(b, t, d).astype(x.dtype)
